# revision 53
# baseline (speedup 1.0000x reference)
"""Trainium2 Bass kernel for Transformer-XL-style relative-position attention.

Problem (per reference):
  T=512 tokens, B=8 batch, D=512 model dim, H=8 heads, DH=64.
  energy = (q+u)@k^T + (q+v)@rpe^T(rel) ; rpe = sinusoidal(i-j) @ W_pos
  softmax over j (diag masked), out = (attn@v) @ W_out + b_out.

Strategy:
  - Data parallel over batch: core b computes batch element b end-to-end.
    No collectives needed.
  - The (t,t,d) rpe tensor is never materialized. Using
    sin((i-j)f) = sin(if)cos(jf) - cos(if)sin(jf) (and the cos analog),
    the BD term factorizes exactly into plain matmuls:
      P^T   = W_pos_h^T @ (q+v)^T            (per head, contraction 64)
      C1    = sin(if).P_sin + cos(if).P_cos  (elementwise, DVE)
      C2    = sin(if).P_cos - cos(if).P_sin
      BD^T  = G^T.T @ [C1;C2]  where G = [cos(jf) | sin(jf)] is constant.
  - The BD matmuls run in fp8e4 DoubleRow mode (2 k-tiles per
    instruction): half the PE column time of the bf16 version. C tiles
    are written fp8 directly by the DVE/GpSimd modulation ops.
  - Everything runs in feature-major ("transposed") layout (j on
    partitions): energies accumulate in PSUM together with the (q+u)k
    term and a single merged -BIG diagonal-mask matmul (strided 3-D
    output AP covers both j-tiles); one wide exp pass per j-pair
    produces attn^T.
  - V carries 64 ones-columns per head so the attn@v matmul emits the
    softmax denominator replicated on partitions 64:128; 1/den on DVE
    (reciprocal); one tensor_tensor divide.
  - Warm-up matmuls on a memset scratch tile run while the input DMAs
    land, ramping the PE DVFS clock before real work arrives. Inputs
    stream in dependency order in per-tile chunks.
  - bf16 matmul inputs (fp8 for BD), fp32 PSUM accumulation, bf16 out.
"""

import sys

sys.path.insert(0, "/opt/trn_rl_repo")

import numpy as np
import ml_dtypes

T, B, D, H = 512, 8, 512, 8
DH = D // H
HALF = D // 2
NT = T // 128          # 4 token tiles
ND = D // 128          # 4 feature tiles
NEG_BIG = -30000.0

BF16 = ml_dtypes.bfloat16
F8 = ml_dtypes.float8_e4m3

_CACHE = {}


def _patch_tile_drain():
    """walrus in this image rejects >1 sync-waits on one TPB_CTRL drain;
    split the TileContext tail-drain waits across several drains."""
    import concourse.tile as tile
    import concourse.mybir as mybir

    if getattr(tile.TileContext, "_drain_patched", False):
        return

    def _drain_and_barrier(self, tick_clock, wait_clock):
        from concourse.vector_clock import ScopedClock

        nc = self.nc
        drain_inst = nc.sync.drain()
        wait_clock.add_sem_waits(
            drain_inst.ins, ScopedClock({None: tick_clock.global_clock})
        )
        si = drain_inst.ins.sync_info
        waits = list(si.on_wait or [])
        if len(waits) > 1:
            si.on_wait[:] = waits[:1]
            for w in waits[1:]:
                extra = nc.sync.drain()
                extra.ins.sync_info = mybir.SyncInfo(on_wait=[w], on_update=[])

        nc.all_engine_barrier()
        assert self.sems is not None
        popped = nc._tile_sem_poison_stack.pop()
        assert popped is self._sem_poison
        nc.clear_and_free_semaphores(list(self.sems.allocated().values()))
        nc.all_engine_barrier()

    tile.TileContext._drain_and_barrier = _drain_and_barrier
    tile.TileContext._drain_patched = True


def _split_multi_waits(nc, limit=1):
    """This walrus build rejects >limit sync-waits on one instruction;
    hoist extra waits onto same-engine NoOp carriers placed just before."""
    import concourse.mybir as mybir

    ctr = [0]
    for f in nc.m.functions:
        for blk in f.blocks:
            new_insts = []
            for inst in blk.instructions:
                si = inst.sync_info
                waits = list(si.on_wait) if si and si.on_wait else []
                if len(waits) > limit:
                    for i in range(limit, len(waits), limit):
                        ctr[0] += 1
                        nop = mybir.InstNoOp(
                            name=f"waitnop{ctr[0]}", ins=[], outs=[]
                        )
                        nop.engine = inst.engine
                        nop.sync_info = mybir.SyncInfo(
                            on_wait=waits[i : i + limit], on_update=[]
                        )
                        new_insts.append(nop)
                    si.on_wait[:] = waits[:limit]
                new_insts.append(inst)
            blk.instructions[:] = new_insts


def _build():
    import concourse.bass as bass
    import concourse.mybir as mybir
    import concourse.tile as tile

    _patch_tile_drain()

    f32 = mybir.dt.float32
    bf16 = mybir.dt.bfloat16
    fp8 = mybir.dt.float8e4
    AF = mybir.ActivationFunctionType
    DR = mybir.MatmulPerfMode.DoubleRow

    nc = bass.Bass("TRN2", target_bir_lowering=True, debug=False, num_devices=B)

    with tile.TileContext(nc) as tc:
        # ---- DRAM parameters, packed into one tensor loaded as a few
        # chunked DMAs (each trigger costs ~650ns serialized on its
        # queue; triggers are split across the sync and scalar HWDGE
        # queues). Column layout (bf16 columns):
        #   A0 [    0: 1050): wqn0 | wqn4 | sml-as-bf16(26)
        #   A1 [ 1050: 3098): wqn1 | wqn5 | wqn2 | wqn6
        #   A1b[ 3098: 4122): wqn3 | wqn7
        #   A2 [ 4122: 6170): wpT(2048)
        #   A3 [ 6170: 8218): gt(2048)
        #   B  [ 8218: 9626): gt8-as-bf16(1024) | msk(384)
        #   C  [ 9626:11674): wqv(2048)
        #   D  [11674:13722): wout(2048)
        PCOLS = 13722
        xT_d = nc.dram_tensor("xT", [128, ND * T], bf16, kind="ExternalInput")
        par_d = nc.dram_tensor("par", [128, PCOLS], bf16, kind="ExternalInput")
        out_d = nc.dram_tensor("out", [128, ND * T], bf16, kind="ExternalOutput")

        # ---- static SBUF tiles -----------------------------------------
        with tc.tile_pool(name="static", bufs=1) as sp:
            warm = sp.tile([128, 640], bf16, name="warm")
            xtw = sp.tile([128, 4 * T], bf16, name="xtw")
            pA0 = sp.tile([128, 1050], bf16, name="pA0")
            pA1 = sp.tile([128, 3072], bf16, name="pA1")
            pA2 = sp.tile([128, 2048], bf16, name="pA2")
            pA3 = sp.tile([128, 2048], bf16, name="pA3")
            pB = sp.tile([128, 1408], bf16, name="pB")
            pC = sp.tile([128, 2048], bf16, name="pC")
            pD = sp.tile([128, 2048], bf16, name="pD")
            mb = sp.tile([128, 1], f32, name="mb")

            quT = [sp.tile([128, T], bf16, name=f"quT{i}") for i in range(ND)]
            qvT = [sp.tile([128, T], bf16, name=f"qvT{i}") for i in range(ND)]
            kT = [sp.tile([128, T], bf16, name=f"kT{i}") for i in range(ND)]
            vsb = [sp.tile([128, 8 * 128], bf16, name=f"v{i}") for i in range(NT)]
            avn = [sp.tile([128, T], bf16, name=f"avn{i}") for i in range(ND)]
            outw = sp.tile([128, ND * T], bf16, name="outw")

            # input DMAs in dependency order; fine-grained chunks so the
            # pipeline starts as soon as the first pieces land
            nc.sync.dma_start(xtw[:, 0:T], xT_d[:, 0:T])
            nc.sync.dma_start(pA0[:], par_d[:, 0:1050])
            nc.sync.dma_start(xtw[:, T:4 * T], xT_d[:, T:4 * T])
            nc.sync.dma_start(pA1[:, 0:2048], par_d[:, 1050:3098])
            nc.sync.dma_start(pA1[:, 2048:3072], par_d[:, 3098:4122])
            nc.sync.dma_start(pA2[:], par_d[:, 4122:6170])
            nc.sync.dma_start(pA3[:], par_d[:, 6170:8218])
            nc.sync.dma_start(pB[:], par_d[:, 8218:9626])
            nc.sync.dma_start(pC[:], par_d[:, 9626:11674])
            nc.sync.dma_start(pD[:], par_d[:, 11674:13722])

            def xt(dt):
                return xtw[:, dt * T:(dt + 1) * T]

            wqn = {}
            for i, nt_ in enumerate((0, 4)):
                wqn[nt_] = pA0[:, i * 512:(i + 1) * 512]
            for i, nt_ in enumerate((1, 5, 2, 6, 3, 7)):
                wqn[nt_] = pA1[:, i * 512:(i + 1) * 512]
            wqv = pC[:, 0:2048]
            wptw = pA2[:, 0:2048]
            gtw = pA3[:, 0:2048]
            gt8w = pB[:, 0:1024].bitcast(fp8)
            msk = pB[:, 1024:1408]
            sml = pA0[:, 1024:1050].bitcast(f32)
            woutw = pD[:, 0:2048]

            sinw = gtw[:, 2 * T:4 * T]
            cosw = gtw[:, 0:2 * T]

            def wpT(dt, c0, c1):
                return wptw[:, dt * D + c0:dt * D + c1]

            def wout(dt, c0, c1):
                return woutw[:, dt * D + c0:dt * D + c1]

            def gt8(a, jt):
                # [128, 2, 128] DoubleRow stationary for trig bank a, j-tile jt
                return gt8w.rearrange(
                    "p (a t j) -> p a t j", a=2, t=2
                )[:, a, :, jt * 128:(jt + 1) * 128]

            dneg = msk[:, 0:128]
            eye2 = msk[:, 128:384]
            posu = [sml[:, i:i + 1] for i in range(ND)]
            posv = [sml[:, 4 + i:5 + i] for i in range(ND)]
            bout = [sml[:, 8 + i:9 + i] for i in range(ND)]
            ltau = sml[:, 12:13]

            # m = exp(ltau) * DH^-0.5  (log-space fold of the 1/8 scale)
            lnm = sp.tile([128, 1], f32, name="lnm")
            nc.vector.memset(lnm[:], float(np.log(DH ** -0.5)))
            nc.scalar.activation(mb[:], ltau, AF.Exp, bias=lnm[:], scale=1.0)

            with (
                tc.tile_pool(name="work", bufs=2) as wk,
                tc.tile_pool(name="ps", bufs=2, space="PSUM") as ps,
            ):
                # shared PSUM tags: acc (2x1 bank), p (1x2), e (2x2) = 8
                ps_p = ps_e = ps_av = ps

                # ---- PE warm-up: ramp the DVFS clock while inputs land.
                # warm is memset (no DMA dep) so these fire right after the
                # preamble barriers and keep the PE busy through the input
                # DMA window; outputs go to an e-tag PSUM tile nothing reads.
                nc.gpsimd.memset(warm[:], 0.0)
                wacc = ps.tile([128, T], f32, name="warm_ps", tag="e")
                for _ in range(9):
                    nc.tensor.matmul(
                        wacc[:],
                        warm[:, 0:128],
                        warm[:, 128:640],
                        start=True,
                        stop=True,
                        skip_group_check=True,
                    )

                def emit_qkv(ntile):
                    acc = ps.tile([128, T], f32, name="qkv_ps", tag="acc")
                    for dt in range(ND):
                        nc.tensor.matmul(
                            acc[:],
                            wqn[ntile][:, dt * 128:(dt + 1) * 128],
                            xt(dt),
                            start=(dt == 0),
                            stop=(dt == ND - 1),
                        )
                    if ntile < 4:
                        nc.scalar.activation(
                            quT[ntile][:], acc[:], AF.Identity,
                            bias=posu[ntile], scale=1.0,
                        )
                        nc.vector.tensor_scalar_add(
                            qvT[ntile][:], acc[:], posv[ntile]
                        )
                    else:
                        nc.scalar.copy(kT[ntile - 4][:], acc[:])

                def emit_v(it):
                    acc = ps.tile([128, D], f32, name="v_ps", tag="acc")
                    for dt in range(ND):
                        vrhs = wqv.rearrange(
                            "p (nt dt c) -> p nt dt c", nt=4, dt=4
                        )[:, :, dt, :]
                        nc.tensor.matmul(
                            acc[:],
                            xt(dt)[:, it * 128:(it + 1) * 128],
                            vrhs,
                            start=(dt == 0),
                            stop=(dt == ND - 1),
                        )
                    vview = vsb[it][:].rearrange("p (h c) -> p h c", c=128)
                    nc.scalar.copy(
                        vview[:, :, 0:64],
                        acc[:].rearrange("p (h c) -> p h c", c=64),
                    )
                    nc.gpsimd.memset(vview[:, :, 64:128], 1.0)

                # ---- per-head pipeline, software-pipelined ------------
                # P/modulation for head h+3 are interleaved into head h's
                # energy/attention matmuls so the PE never waits on the
                # DVE/GpSimd modulation chain.
                st = {}

                def emit_p_pair(h, gp):
                    hd_tile = h // 2
                    hd_off = (h % 2) * 64
                    qv_h = qvT[hd_tile][hd_off:hd_off + 64, :]
                    d = st.setdefault(h, {})
                    pacc = ps_p.tile([128, 2 * T], f32, name=f"p_ps{gp}_{h}",
                                     tag="p", bufs=1)
                    for g2 in range(2):
                        g = gp * 2 + g2
                        nc.tensor.matmul(
                            pacc[:, g2 * T:(g2 + 1) * T],
                            wpT(hd_tile, g * 128, (g + 1) * 128)[
                                hd_off:hd_off + 64, :
                            ],
                            qv_h,
                            start=True,
                            stop=True,
                            skip_group_check=True,
                        )
                    ptw = wk.tile([128, 2 * T], bf16, name=f"ptw{gp}_{h}",
                                  tag=f"ptw{gp}", bufs=4)
                    d[f"ptw{gp}"] = ptw
                    if gp == 0:
                        nc.scalar.copy(ptw[:], pacc[:])
                    else:
                        nc.vector.tensor_copy(ptw[:], pacc[:])

                def emit_mods(h):
                    # C1 = sin.Psin + cos.Pcos ; C2 = sin.Pcos - cos.Psin
                    # (fp8 outputs, DoubleRow rhs layout [p, f-half, i])
                    d = st[h]
                    psin, pcos = d["ptw0"], d["ptw1"]
                    ctw = [
                        wk.tile([128, 2 * T], fp8, name=f"ctw{g}_{h}",
                                tag=f"ctw{g}", bufs=4)
                        for g in range(2)
                    ]
                    d["ctw"] = ctw
                    ta = wk.tile([128, 2 * T], bf16, name=f"ta{h}", tag="ta",
                                 bufs=4)
                    tb = wk.tile([128, 2 * T], bf16, name=f"tb{h}", tag="tb",
                                 bufs=4)
                    nc.vector.tensor_mul(ta[:], sinw, psin[:])
                    nc.vector.tensor_mul(tb[:], cosw, pcos[:])
                    nc.vector.tensor_add(ctw[0][:], ta[:], tb[:])
                    ta2 = wk.tile([128, 2 * T], bf16, name=f"ta2{h}", tag="ta2",
                                  bufs=4)
                    tb2 = wk.tile([128, 2 * T], bf16, name=f"tb2{h}", tag="tb2",
                                  bufs=4)
                    nc.vector.tensor_mul(ta2[:], sinw, pcos[:])
                    nc.vector.tensor_mul(tb2[:], cosw, psin[:])
                    nc.gpsimd.tensor_sub(ctw[1][:], ta2[:], tb2[:])

                emit_qkv(0)
                emit_qkv(4)
                emit_p_pair(0, 0)
                emit_p_pair(0, 1)
                emit_qkv(1)
                emit_qkv(5)
                emit_mods(0)
                emit_p_pair(1, 0)
                emit_p_pair(1, 1)
                emit_qkv(2)
                emit_qkv(6)
                emit_mods(1)
                emit_p_pair(2, 0)
                emit_p_pair(2, 1)
                emit_qkv(3)
                emit_qkv(7)
                emit_mods(2)

                def emit_av_pair(h, attnTw, avacc, jts):
                    for jt in jts:
                        nc.tensor.matmul(
                            avacc[:],
                            vsb[jt][:, h * 128:(h + 1) * 128],
                            attnTw[jt // 2][:, (jt % 2) * T:(jt % 2 + 1) * T],
                            start=(jt == 0),
                            stop=(jt == NT - 1),
                            skip_group_check=True,
                        )

                for h in range(H):
                    hd_tile = h // 2
                    hd_off = (h % 2) * 64
                    qu_h = quT[hd_tile][hd_off:hd_off + 64, :]
                    ctw = st[h]["ctw"]

                    # energy^T per wide j-pair; one wide exp per pair
                    attnTw = [
                        wk.tile([128, 2 * T], bf16, name=f"atw{j}_{h}",
                                tag=f"atw{j}")
                        for j in range(2)
                    ]
                    for jp in range(2):
                        eacc = ps_e.tile([128, 2 * T], f32, name="e_ps", tag="e",
                                          bufs=2)
                        # K parts first (only need quT), then the fp8
                        # DoubleRow BD accumulations, merged mask last
                        for j2 in range(2):
                            jt = jp * 2 + j2
                            nc.tensor.matmul(
                                eacc[:, j2 * T:(j2 + 1) * T],
                                kT[hd_tile][hd_off:hd_off + 64,
                                            jt * 128:(jt + 1) * 128],
                                qu_h,
                                start=True,
                                stop=False,
                                skip_group_check=True,
                            )
                        for a in range(2):
                            crhs = ctw[a][:].rearrange(
                                "p (t n) -> p t n", t=2
                            )
                            for j2 in range(2):
                                jt = jp * 2 + j2
                                nc.tensor.matmul(
                                    eacc[:, j2 * T:(j2 + 1) * T],
                                    gt8(a, jt),
                                    crhs,
                                    start=False,
                                    stop=False,
                                    perf_mode=DR,
                                    skip_group_check=True,
                                )
                        # merged diag mask: one matmul, strided 3-D out
                        # covering cols {jp*256.. , 640+jp*256..}
                        from concourse.ap import AP

                        eap = eacc[:]
                        mout = AP(
                            eap.tensor,
                            eap.offset + jp * 256,
                            [[2 * T, 128], [640, 2], [1, 128]],
                        )
                        nc.tensor.matmul(
                            mout,
                            dneg,
                            eye2[:].rearrange("p (t c) -> p t c", t=2),
                            start=False,
                            stop=True,
                            skip_group_check=True,
                        )
                        nc.scalar.activation(
                            attnTw[jp][:], eacc[:], AF.Exp, bias=0.0, scale=mb[:],
                        )

                    # attn@v feature-major; ones rows give the denominator
                    # replicated on partitions 64:128. P matmuls for head
                    # h+3 are interleaved to cover the exp latency.
                    if h == 0:
                        for it_ in range(NT):
                            emit_v(it_)
                    avacc = ps_av.tile([128, T], f32, name="av_ps", tag="acc")
                    if h + 3 < H:
                        emit_p_pair(h + 3, 0)
                    emit_av_pair(h, attnTw, avacc, [0, 1])
                    if h + 3 < H:
                        emit_p_pair(h + 3, 1)
                    emit_av_pair(h, attnTw, avacc, [2, 3])
                    if h + 3 < H:
                        emit_mods(h + 3)
                    st.pop(h - 1, None)

                    lnden = wk.tile([64, T], f32, name="lnden", tag="lnden")
                    rdb = wk.tile([64, T], f32, name="rdb", tag="rdb")
                    nc.scalar.activation(
                        lnden[:], avacc[64:128, :], AF.Ln, bias=0.0, scale=1.0
                    )
                    nc.scalar.activation(
                        rdb[:], lnden[:], AF.Exp, bias=0.0, scale=-1.0
                    )
                    nc.vector.tensor_mul(
                        avn[hd_tile][hd_off:hd_off + 64, :],
                        avacc[0:64, :],
                        rdb[:],
                    )

                    if h == 5:
                        # W_out partials for ot 0,1 over d-tiles 0,1
                        # (avn[0..2] are complete after this head; dt=2
                        # is deferred to h==6 to fill that head's gap)
                        st["oacc01"] = ps.tile(
                            [128, 2 * T], f32, name="oacc01", tag="p", bufs=1
                        )
                        for dt in range(2):
                            for o2 in range(2):
                                nc.tensor.matmul(
                                    st["oacc01"][:, o2 * T:(o2 + 1) * T],
                                    wout(dt, o2 * 128, (o2 + 1) * 128),
                                    avn[dt][:],
                                    start=(dt == 0),
                                    stop=False,
                                    skip_group_check=True,
                                )
                    if h == 6:
                        for o2 in range(2):
                            nc.tensor.matmul(
                                st["oacc01"][:, o2 * T:(o2 + 1) * T],
                                wout(2, o2 * 128, (o2 + 1) * 128),
                                avn[2][:],
                                start=False,
                                stop=False,
                                skip_group_check=True,
                            )
                    if h == 7:
                        st["oacc23"] = ps.tile(
                            [128, 2 * T], f32, name="oacc23", tag="e", bufs=2
                        )
                        for dt in range(3):
                            for o2 in range(2):
                                nc.tensor.matmul(
                                    st["oacc23"][:, o2 * T:(o2 + 1) * T],
                                    wout(dt, (2 + o2) * 128, (3 + o2) * 128),
                                    avn[dt][:],
                                    start=(dt == 0),
                                    stop=False,
                                    skip_group_check=True,
                                )
                # zero-contribution matmuls (warm is memset 0) keep the PE
                # clock at max p-state through the last head's den chain
                for _ in range(4):
                    nc.tensor.matmul(
                        st["oacc23"][:, 0:T],
                        warm[:, 0:128],
                        warm[:, 128:640],
                        start=False,
                        stop=False,
                        skip_group_check=True,
                    )

                # ---- output projection: finish dt=3 and write out -----
                for pair, tname in ((0, "oacc01"), (1, "oacc23")):
                    oacc = st[tname]
                    for o2 in range(2):
                        ot = pair * 2 + o2
                        nc.tensor.matmul(
                            oacc[:, o2 * T:(o2 + 1) * T],
                            wout(3, ot * 128, (ot + 1) * 128),
                            avn[3][:],
                            start=False,
                            stop=True,
                            skip_group_check=True,
                        )
                        nc.vector.tensor_scalar_add(
                            outw[:, ot * T:(ot + 1) * T],
                            oacc[:, o2 * T:(o2 + 1) * T],
                            bout[ot],
                        )
                    nc.sync.dma_start(
                        out_d[:, pair * 2 * T:(pair + 1) * 2 * T],
                        outw[:, pair * 2 * T:(pair + 1) * 2 * T],
                    )

    _split_multi_waits(nc)
    return nc


def _nmajor(a):
    """(512, 1536) -> (128, 12*4*128): [p, nt*512 + dt*128 + c]
    = a[dt*128 + p, nt*128 + c]."""
    out = np.empty((128, 12, 4, 128), a.dtype)
    for nt in range(12):
        for dt in range(4):
            out[:, nt, dt, :] = a[dt * 128:(dt + 1) * 128,
                                  nt * 128:(nt + 1) * 128]
    return np.ascontiguousarray(out.reshape(128, 6144))


def _coalesce(a):
    """(128*ND, W) -> (128, ND*W): [p, dt*W + c] = a[dt*128 + p, c]."""
    n, w = a.shape
    nd = n // 128
    return np.ascontiguousarray(
        a.reshape(nd, 128, w).transpose(1, 0, 2).reshape(128, nd * w)
    )


def _host_constants():
    freqs = np.exp(
        -np.log(10000.0) * np.arange(HALF, dtype=np.float32) / HALF
    )
    idx = np.arange(T, dtype=np.float32)
    ang = np.outer(freqs, idx)  # (HALF, T)
    sing = np.sin(ang).astype(np.float32)
    cosg = np.cos(ang).astype(np.float32)
    gt = _coalesce(np.concatenate([cosg, sing], axis=0)).astype(BF16)
    # fp8 DoubleRow pairing: gt8[p, a*1024 + t*512 + j] = trig_a[t*128+p, j]
    gt8 = np.empty((128, 2, 2, T), np.float32)
    for t in range(2):
        gt8[:, 0, t, :] = cosg[t * 128:(t + 1) * 128]
        gt8[:, 1, t, :] = sing[t * 128:(t + 1) * 128]
    gt8 = np.ascontiguousarray(gt8.reshape(128, 2048)).astype(F8)
    eye = np.eye(128, dtype=np.float32)
    msk = np.concatenate([NEG_BIG * eye, eye, eye], axis=1).astype(BF16)
    return gt, gt8, msk


def kernel(x, W_qkv, W_pos, pos_u, pos_v, W_out, b_out, log_one_div_by_tau):
    from concourse import bass_utils

    if "nc" not in _CACHE:
        _CACHE["nc"] = _build()
        _CACHE["consts"] = _host_constants()
    nc = _CACHE["nc"]
    gt, gt8, msk = _CACHE["consts"]

    x = np.asarray(x, np.float32)
    sml = np.zeros((128, 13), np.float32)
    for i in range(ND):
        sml[:, i] = np.asarray(pos_u, np.float32).reshape(D)[
            i * 128:(i + 1) * 128
        ]
        sml[:, 4 + i] = np.asarray(pos_v, np.float32).reshape(D)[
            i * 128:(i + 1) * 128
        ]
        sml[:, 8 + i] = np.asarray(b_out, np.float32).reshape(D)[
            i * 128:(i + 1) * 128
        ]
    sml[:, 12] = np.float32(np.asarray(log_one_div_by_tau).reshape(-1)[0])

    wqkv = _nmajor(np.asarray(W_qkv, np.float32)).astype(BF16)
    wpt = _coalesce(
        np.ascontiguousarray(np.asarray(W_pos, np.float32).T)
    ).astype(BF16)
    woutc = _coalesce(np.asarray(W_out, np.float32)).astype(BF16)

    def wqnb(nt):
        return wqkv[:, nt * 512:(nt + 1) * 512]

    par = np.concatenate(
        [
            wqnb(0), wqnb(4),                                  # A0
            np.ascontiguousarray(sml).view(BF16),
            wqnb(1), wqnb(5), wqnb(2), wqnb(6),                # A1
            wqnb(3), wqnb(7),                                  # A1b
            wpt,                                               # A2
            gt,                                                # A3
            np.ascontiguousarray(gt8).view(BF16),              # B
            msk,
            wqkv[:, 4096:6144],                                # C
            woutc,                                             # D
        ],
        axis=1,
    )
    assert par.shape == (128, 13722), par.shape

    in_maps = []
    for b in range(B):
        in_maps.append({
            "par": par,
            "xT": _coalesce(
                np.ascontiguousarray(x[:, b, :].T)
            ).astype(BF16),
        })

    _CACHE["last_in_maps"] = in_maps
    res = bass_utils.run_bass_kernel_spmd(nc, in_maps, core_ids=list(range(B)))
    out = np.empty((T, B, D), np.float32)
    for b in range(B):
        o = np.asarray(res.results[b]["out"], np.float32)  # (128, ND*T)
        out[:, b, :] = (
            o.reshape(128, ND, T).transpose(1, 0, 2).reshape(D, T).T
        )
    return out


# revision 54
# speedup vs baseline: 1.1816x; 1.1816x over previous
"""Trainium2 Bass kernel for Transformer-XL-style relative-position attention.

Problem (per reference):
  T=512 tokens, B=8 batch, D=512 model dim, H=8 heads, DH=64.
  energy = (q+u)@k^T + (q+v)@rpe^T(rel) ; rpe = sinusoidal(i-j) @ W_pos
  softmax over j (diag masked), out = (attn@v) @ W_out + b_out.

Strategy:
  - Data parallel over batch: core b computes batch element b end-to-end.
    No collectives needed.
  - The (t,t,d) rpe tensor is never materialized. Using
    sin((i-j)f) = sin(if)cos(jf) - cos(if)sin(jf) (and the cos analog),
    the BD term factorizes exactly into plain matmuls:
      P^T   = W_pos_h^T @ (q+v)^T            (per head, contraction 64)
      C1    = sin(if).P_sin + cos(if).P_cos  (elementwise, DVE)
      C2    = sin(if).P_cos - cos(if).P_sin
      BD^T  = G^T.T @ [C1;C2]  where G = [cos(jf) | sin(jf)] is constant.
  - The BD matmuls run in fp8e4 DoubleRow mode (2 k-tiles per
    instruction): half the PE column time of the bf16 version. C tiles
    are written fp8 directly by the DVE/GpSimd modulation ops.
  - Everything runs in feature-major ("transposed") layout (j on
    partitions): energies accumulate in PSUM together with the (q+u)k
    term and a single merged -BIG diagonal-mask matmul (strided 3-D
    output AP covers both j-tiles); one wide exp pass per j-pair
    produces attn^T.
  - V carries 64 ones-columns per head so the attn@v matmul emits the
    softmax denominator replicated on partitions 64:128; 1/den on DVE
    (reciprocal); one tensor_tensor divide.
  - Warm-up matmuls on a memset scratch tile run while the input DMAs
    land, ramping the PE DVFS clock before real work arrives. Inputs
    stream in dependency order in per-tile chunks.
  - bf16 matmul inputs (fp8 for BD), fp32 PSUM accumulation, bf16 out.
"""

import sys

sys.path.insert(0, "/opt/trn_rl_repo")

import numpy as np
import ml_dtypes

T, B, D, H = 512, 8, 512, 8
DH = D // H
HALF = D // 2
NT = T // 128          # 4 token tiles
ND = D // 128          # 4 feature tiles
NEG_BIG = -30000.0

BF16 = ml_dtypes.bfloat16
F8 = ml_dtypes.float8_e4m3

_CACHE = {}


def _patch_tile_drain():
    """walrus in this image rejects >1 sync-waits on one TPB_CTRL drain;
    split the TileContext tail-drain waits across several drains."""
    import concourse.tile as tile
    import concourse.mybir as mybir

    if getattr(tile.TileContext, "_drain_patched", False):
        return

    def _drain_and_barrier(self, tick_clock, wait_clock):
        from concourse.vector_clock import ScopedClock

        nc = self.nc
        drain_inst = nc.sync.drain()
        wait_clock.add_sem_waits(
            drain_inst.ins, ScopedClock({None: tick_clock.global_clock})
        )
        si = drain_inst.ins.sync_info
        waits = list(si.on_wait or [])
        if len(waits) > 1:
            si.on_wait[:] = waits[:1]
            for w in waits[1:]:
                extra = nc.sync.drain()
                extra.ins.sync_info = mybir.SyncInfo(on_wait=[w], on_update=[])

        nc.all_engine_barrier()
        assert self.sems is not None
        popped = nc._tile_sem_poison_stack.pop()
        assert popped is self._sem_poison
        nc.clear_and_free_semaphores(list(self.sems.allocated().values()))
        nc.all_engine_barrier()

    tile.TileContext._drain_and_barrier = _drain_and_barrier
    tile.TileContext._drain_patched = True


def _split_multi_waits(nc, limit=1):
    """This walrus build rejects >limit sync-waits on one instruction;
    hoist extra waits onto same-engine NoOp carriers placed just before."""
    import concourse.mybir as mybir

    ctr = [0]
    for f in nc.m.functions:
        for blk in f.blocks:
            new_insts = []
            for inst in blk.instructions:
                si = inst.sync_info
                waits = list(si.on_wait) if si and si.on_wait else []
                if len(waits) > limit:
                    for i in range(limit, len(waits), limit):
                        ctr[0] += 1
                        nop = mybir.InstNoOp(
                            name=f"waitnop{ctr[0]}", ins=[], outs=[]
                        )
                        nop.engine = inst.engine
                        nop.sync_info = mybir.SyncInfo(
                            on_wait=waits[i : i + limit], on_update=[]
                        )
                        new_insts.append(nop)
                    si.on_wait[:] = waits[:limit]
                new_insts.append(inst)
            blk.instructions[:] = new_insts


def _build():
    import concourse.bass as bass
    import concourse.mybir as mybir
    import concourse.tile as tile

    _patch_tile_drain()

    f32 = mybir.dt.float32
    bf16 = mybir.dt.bfloat16
    fp8 = mybir.dt.float8e4
    AF = mybir.ActivationFunctionType
    DR = mybir.MatmulPerfMode.DoubleRow

    nc = bass.Bass("TRN2", target_bir_lowering=True, debug=False, num_devices=B)

    with tile.TileContext(nc) as tc:
        # ---- DRAM parameters, packed into one tensor loaded as a few
        # chunked DMAs (each trigger costs ~650ns serialized on its
        # queue; triggers are split across the sync and scalar HWDGE
        # queues). Column layout (bf16 columns):
        #   A0 [    0: 1050): wqn0 | wqn4 | sml-as-bf16(26)
        #   A1 [ 1050: 3098): wqn1 | wqn5 | wqn2 | wqn6
        #   A1b[ 3098: 4122): wqn3 | wqn7
        #   A2 [ 4122: 6170): wpT(2048)
        #   A3 [ 6170: 8218): gt(2048)
        #   B  [ 8218: 9626): gt8-as-bf16(1024) | msk(384)
        #   C  [ 9626:11674): wqv(2048)
        #   D  [11674:13722): wout(2048)
        PCOLS = 13722
        xT_d = nc.dram_tensor("xT", [128, ND * T], bf16, kind="ExternalInput")
        par_d = nc.dram_tensor("par", [128, PCOLS], bf16, kind="ExternalInput")
        out_d = nc.dram_tensor("out", [128, ND * T], bf16, kind="ExternalOutput")

        # ---- static SBUF tiles -----------------------------------------
        with tc.tile_pool(name="static", bufs=1) as sp:
            warm = sp.tile([128, 640], bf16, name="warm")
            xtw = sp.tile([128, 4 * T], bf16, name="xtw")
            pA0 = sp.tile([128, 1050], bf16, name="pA0")
            pA1 = sp.tile([128, 3072], bf16, name="pA1")
            pA2 = sp.tile([128, 2048], bf16, name="pA2")
            pA3 = sp.tile([128, 2048], bf16, name="pA3")
            pB = sp.tile([128, 1408], bf16, name="pB")
            pC = sp.tile([128, 2048], bf16, name="pC")
            pD = sp.tile([128, 2048], bf16, name="pD")
            mb = sp.tile([128, 1], f32, name="mb")

            quT = [sp.tile([128, T], bf16, name=f"quT{i}") for i in range(ND)]
            qvT = [sp.tile([128, T], bf16, name=f"qvT{i}") for i in range(ND)]
            kT = [sp.tile([128, T], bf16, name=f"kT{i}") for i in range(ND)]
            vsb = [sp.tile([128, 8 * 128], bf16, name=f"v{i}") for i in range(NT)]
            avn = [sp.tile([128, T], bf16, name=f"avn{i}") for i in range(ND)]
            outw = sp.tile([128, ND * T], bf16, name="outw")

            # input DMAs in dependency order; fine-grained chunks so the
            # pipeline starts as soon as the first pieces land
            nc.sync.dma_start(xtw[:, 0:T], xT_d[:, 0:T])
            nc.sync.dma_start(pA0[:], par_d[:, 0:1050])
            nc.sync.dma_start(xtw[:, T:4 * T], xT_d[:, T:4 * T])
            nc.sync.dma_start(pA1[:, 0:2048], par_d[:, 1050:3098])
            nc.sync.dma_start(pA1[:, 2048:3072], par_d[:, 3098:4122])
            nc.sync.dma_start(pA2[:], par_d[:, 4122:6170])
            nc.sync.dma_start(pA3[:], par_d[:, 6170:8218])
            nc.sync.dma_start(pB[:], par_d[:, 8218:9626])
            nc.sync.dma_start(pC[:], par_d[:, 9626:11674])
            nc.sync.dma_start(pD[:], par_d[:, 11674:13722])

            def xt(dt):
                return xtw[:, dt * T:(dt + 1) * T]

            wqn = {}
            for i, nt_ in enumerate((0, 4)):
                wqn[nt_] = pA0[:, i * 512:(i + 1) * 512]
            for i, nt_ in enumerate((1, 5, 2, 6, 3, 7)):
                wqn[nt_] = pA1[:, i * 512:(i + 1) * 512]
            wqv = pC[:, 0:2048]
            wptw = pA2[:, 0:2048]
            gtw = pA3[:, 0:2048]
            gt8w = pB[:, 0:1024].bitcast(fp8)
            msk = pB[:, 1024:1408]
            sml = pA0[:, 1024:1050].bitcast(f32)
            woutw = pD[:, 0:2048]

            sinw = gtw[:, 2 * T:4 * T]
            cosw = gtw[:, 0:2 * T]

            def wpT(dt, c0, c1):
                return wptw[:, dt * D + c0:dt * D + c1]

            def wout(dt, c0, c1):
                return woutw[:, dt * D + c0:dt * D + c1]

            def gt8(a, jt):
                # [128, 2, 128] DoubleRow stationary for trig bank a, j-tile jt
                return gt8w.rearrange(
                    "p (a t j) -> p a t j", a=2, t=2
                )[:, a, :, jt * 128:(jt + 1) * 128]

            dneg = msk[:, 0:128]
            eye2 = msk[:, 128:384]
            posu = [sml[:, i:i + 1] for i in range(ND)]
            posv = [sml[:, 4 + i:5 + i] for i in range(ND)]
            bout = [sml[:, 8 + i:9 + i] for i in range(ND)]
            ltau = sml[:, 12:13]

            # m = exp(ltau) * DH^-0.5  (log-space fold of the 1/8 scale)
            lnm = sp.tile([128, 1], f32, name="lnm")
            nc.vector.memset(lnm[:], float(np.log(DH ** -0.5)))
            nc.scalar.activation(mb[:], ltau, AF.Exp, bias=lnm[:], scale=1.0)

            with (
                tc.tile_pool(name="work", bufs=2) as wk,
                tc.tile_pool(name="ps", bufs=2, space="PSUM") as ps,
            ):
                # shared PSUM tags: acc (2x1 bank), p (1x2), e (2x2) = 8
                ps_p = ps_e = ps_av = ps

                # ---- PE warm-up: ramp the DVFS clock while inputs land.
                # warm is memset (no DMA dep) so these fire right after the
                # preamble barriers and keep the PE busy through the input
                # DMA window; outputs go to an e-tag PSUM tile nothing reads.
                nc.gpsimd.memset(warm[:], 0.0)
                wacc = ps.tile([128, T], f32, name="warm_ps", tag="e")
                for _ in range(9):
                    nc.tensor.matmul(
                        wacc[:],
                        warm[:, 0:128],
                        warm[:, 128:640],
                        start=True,
                        stop=True,
                        skip_group_check=True,
                    )

                def emit_qkv(ntile):
                    acc = ps.tile([128, T], f32, name="qkv_ps", tag="acc")
                    for dt in range(ND):
                        nc.tensor.matmul(
                            acc[:],
                            wqn[ntile][:, dt * 128:(dt + 1) * 128],
                            xt(dt),
                            start=(dt == 0),
                            stop=(dt == ND - 1),
                        )
                    if ntile < 4:
                        nc.scalar.activation(
                            quT[ntile][:], acc[:], AF.Identity,
                            bias=posu[ntile], scale=1.0,
                        )
                        nc.vector.tensor_scalar_add(
                            qvT[ntile][:], acc[:], posv[ntile]
                        )
                    else:
                        nc.scalar.copy(kT[ntile - 4][:], acc[:])

                def emit_v(it):
                    acc = ps.tile([128, D], f32, name="v_ps", tag="acc")
                    for dt in range(ND):
                        vrhs = wqv.rearrange(
                            "p (nt dt c) -> p nt dt c", nt=4, dt=4
                        )[:, :, dt, :]
                        nc.tensor.matmul(
                            acc[:],
                            xt(dt)[:, it * 128:(it + 1) * 128],
                            vrhs,
                            start=(dt == 0),
                            stop=(dt == ND - 1),
                        )
                    vview = vsb[it][:].rearrange("p (h c) -> p h c", c=128)
                    nc.scalar.copy(
                        vview[:, :, 0:64],
                        acc[:].rearrange("p (h c) -> p h c", c=64),
                    )
                    nc.gpsimd.memset(vview[:, :, 64:128], 1.0)

                # ---- per-head pipeline, software-pipelined ------------
                # P/modulation for head h+3 are interleaved into head h's
                # energy/attention matmuls so the PE never waits on the
                # DVE/GpSimd modulation chain.
                st = {}

                def emit_p_pair(h, gp):
                    hd_tile = h // 2
                    hd_off = (h % 2) * 64
                    qv_h = qvT[hd_tile][hd_off:hd_off + 64, :]
                    d = st.setdefault(h, {})
                    pacc = ps_p.tile([128, 2 * T], f32, name=f"p_ps{gp}_{h}",
                                     tag="p", bufs=1)
                    for g2 in range(2):
                        g = gp * 2 + g2
                        nc.tensor.matmul(
                            pacc[:, g2 * T:(g2 + 1) * T],
                            wpT(hd_tile, g * 128, (g + 1) * 128)[
                                hd_off:hd_off + 64, :
                            ],
                            qv_h,
                            start=True,
                            stop=True,
                            skip_group_check=True,
                        )
                    ptw = wk.tile([128, 2 * T], bf16, name=f"ptw{gp}_{h}",
                                  tag=f"ptw{gp}", bufs=4)
                    d[f"ptw{gp}"] = ptw
                    if gp == 0:
                        nc.scalar.copy(ptw[:], pacc[:])
                    else:
                        nc.vector.tensor_copy(ptw[:], pacc[:])

                def emit_mods(h):
                    # C1 = sin.Psin + cos.Pcos ; C2 = sin.Pcos - cos.Psin
                    # (fp8 outputs, DoubleRow rhs layout [p, f-half, i])
                    d = st[h]
                    psin, pcos = d["ptw0"], d["ptw1"]
                    ctw = [
                        wk.tile([128, 2 * T], fp8, name=f"ctw{g}_{h}",
                                tag=f"ctw{g}", bufs=4)
                        for g in range(2)
                    ]
                    d["ctw"] = ctw
                    ta = wk.tile([128, 2 * T], bf16, name=f"ta{h}", tag="ta",
                                 bufs=4)
                    tb = wk.tile([128, 2 * T], bf16, name=f"tb{h}", tag="tb",
                                 bufs=4)
                    nc.vector.tensor_mul(ta[:], sinw, psin[:])
                    nc.vector.tensor_mul(tb[:], cosw, pcos[:])
                    nc.vector.tensor_add(ctw[0][:], ta[:], tb[:])
                    ta2 = wk.tile([128, 2 * T], bf16, name=f"ta2{h}", tag="ta2",
                                  bufs=4)
                    tb2 = wk.tile([128, 2 * T], bf16, name=f"tb2{h}", tag="tb2",
                                  bufs=4)
                    nc.vector.tensor_mul(ta2[:], sinw, pcos[:])
                    nc.vector.tensor_mul(tb2[:], cosw, psin[:])
                    nc.gpsimd.tensor_sub(ctw[1][:], ta2[:], tb2[:])

                emit_qkv(0)
                emit_qkv(4)
                emit_p_pair(0, 0)
                emit_p_pair(0, 1)
                emit_qkv(1)
                emit_qkv(5)
                emit_mods(0)
                emit_p_pair(1, 0)
                emit_p_pair(1, 1)
                emit_qkv(2)
                emit_qkv(6)
                emit_mods(1)
                emit_p_pair(2, 0)
                emit_p_pair(2, 1)
                emit_qkv(3)
                emit_qkv(7)
                emit_mods(2)

                def emit_av_pair(h, attnTw, avacc, jts):
                    for jt in jts:
                        nc.tensor.matmul(
                            avacc[:],
                            vsb[jt][:, h * 128:(h + 1) * 128],
                            attnTw[jt // 2][:, (jt % 2) * T:(jt % 2 + 1) * T],
                            start=(jt == 0),
                            stop=(jt == NT - 1),
                            skip_group_check=True,
                        )

                for h in range(H):
                    hd_tile = h // 2
                    hd_off = (h % 2) * 64
                    qu_h = quT[hd_tile][hd_off:hd_off + 64, :]
                    ctw = st[h]["ctw"]

                    # energy^T per wide j-pair; one wide exp per pair
                    attnTw = [
                        wk.tile([128, 2 * T], bf16, name=f"atw{j}_{h}",
                                tag=f"atw{j}")
                        for j in range(2)
                    ]
                    for jp in range(2):
                        eacc = ps_e.tile([128, 2 * T], f32, name="e_ps", tag="e",
                                          bufs=2)
                        # K parts first (only need quT), then the fp8
                        # DoubleRow BD accumulations, merged mask last
                        for j2 in range(2):
                            jt = jp * 2 + j2
                            nc.tensor.matmul(
                                eacc[:, j2 * T:(j2 + 1) * T],
                                kT[hd_tile][hd_off:hd_off + 64,
                                            jt * 128:(jt + 1) * 128],
                                qu_h,
                                start=True,
                                stop=False,
                                skip_group_check=True,
                            )
                        for a in range(2):
                            crhs = ctw[a][:].rearrange(
                                "p (t n) -> p t n", t=2
                            )
                            for j2 in range(2):
                                jt = jp * 2 + j2
                                nc.tensor.matmul(
                                    eacc[:, j2 * T:(j2 + 1) * T],
                                    gt8(a, jt),
                                    crhs,
                                    start=False,
                                    stop=False,
                                    perf_mode=DR,
                                    skip_group_check=True,
                                )
                        # merged diag mask: one matmul, strided 3-D out
                        # covering cols {jp*256.. , 640+jp*256..}
                        from concourse.ap import AP

                        eap = eacc[:]
                        mout = AP(
                            eap.tensor,
                            eap.offset + jp * 256,
                            [[2 * T, 128], [640, 2], [1, 128]],
                        )
                        nc.tensor.matmul(
                            mout,
                            dneg,
                            eye2[:].rearrange("p (t c) -> p t c", t=2),
                            start=False,
                            stop=True,
                            skip_group_check=True,
                        )
                        nc.scalar.activation(
                            attnTw[jp][:], eacc[:], AF.Exp, bias=0.0, scale=mb[:],
                        )

                    # attn@v feature-major; ones rows give the denominator
                    # replicated on partitions 64:128. P matmuls for head
                    # h+3 are interleaved to cover the exp latency.
                    if h == 0:
                        for it_ in range(NT):
                            emit_v(it_)
                    avacc = ps_av.tile([128, T], f32, name="av_ps", tag="acc")
                    if h + 3 < H:
                        emit_p_pair(h + 3, 0)
                    emit_av_pair(h, attnTw, avacc, [0, 1])
                    if h + 3 < H:
                        emit_p_pair(h + 3, 1)
                    emit_av_pair(h, attnTw, avacc, [2, 3])
                    if h + 3 < H:
                        emit_mods(h + 3)
                    st.pop(h - 1, None)

                    lnden = wk.tile([64, T], f32, name="lnden", tag="lnden")
                    rdb = wk.tile([64, T], f32, name="rdb", tag="rdb")
                    nc.scalar.activation(
                        lnden[:], avacc[64:128, :], AF.Ln, bias=0.0, scale=1.0
                    )
                    nc.scalar.activation(
                        rdb[:], lnden[:], AF.Exp, bias=0.0, scale=-1.0
                    )
                    nc.vector.tensor_mul(
                        avn[hd_tile][hd_off:hd_off + 64, :],
                        avacc[0:64, :],
                        rdb[:],
                    )

                    if h == 5:
                        # W_out partials for ot 0,1 over d-tiles 0,1
                        # (avn[0..2] are complete after this head; dt=2
                        # is deferred to h==6 to fill that head's gap)
                        st["oacc01"] = ps.tile(
                            [128, 2 * T], f32, name="oacc01", tag="p", bufs=1
                        )
                        for dt in range(2):
                            for o2 in range(2):
                                nc.tensor.matmul(
                                    st["oacc01"][:, o2 * T:(o2 + 1) * T],
                                    wout(dt, o2 * 128, (o2 + 1) * 128),
                                    avn[dt][:],
                                    start=(dt == 0),
                                    stop=False,
                                    skip_group_check=True,
                                )
                    if h == 6:
                        for o2 in range(2):
                            nc.tensor.matmul(
                                st["oacc01"][:, o2 * T:(o2 + 1) * T],
                                wout(2, o2 * 128, (o2 + 1) * 128),
                                avn[2][:],
                                start=False,
                                stop=False,
                                skip_group_check=True,
                            )
                    if h == 7:
                        st["oacc23"] = ps.tile(
                            [128, 2 * T], f32, name="oacc23", tag="e", bufs=2
                        )
                        for dt in range(3):
                            for o2 in range(2):
                                nc.tensor.matmul(
                                    st["oacc23"][:, o2 * T:(o2 + 1) * T],
                                    wout(dt, (2 + o2) * 128, (3 + o2) * 128),
                                    avn[dt][:],
                                    start=(dt == 0),
                                    stop=False,
                                    skip_group_check=True,
                                )
                # zero-contribution matmuls (warm is memset 0) keep the PE
                # clock at max p-state through the last head's den chain
                for _ in range(4):
                    nc.tensor.matmul(
                        st["oacc23"][:, 0:T],
                        warm[:, 0:128],
                        warm[:, 128:640],
                        start=False,
                        stop=False,
                        skip_group_check=True,
                    )

                # ---- output projection: finish dt=3 and write out -----
                for pair, tname in ((0, "oacc01"), (1, "oacc23")):
                    oacc = st[tname]
                    for o2 in range(2):
                        ot = pair * 2 + o2
                        nc.tensor.matmul(
                            oacc[:, o2 * T:(o2 + 1) * T],
                            wout(3, ot * 128, (ot + 1) * 128),
                            avn[3][:],
                            start=False,
                            stop=True,
                            skip_group_check=True,
                        )
                        nc.vector.tensor_scalar_add(
                            outw[:, ot * T:(ot + 1) * T],
                            oacc[:, o2 * T:(o2 + 1) * T],
                            bout[ot],
                        )
                        nc.sync.dma_start(
                            out_d[:, ot * T:(ot + 1) * T],
                            outw[:, ot * T:(ot + 1) * T],
                        )

    _split_multi_waits(nc)
    return nc


def _nmajor(a):
    """(512, 1536) -> (128, 12*4*128): [p, nt*512 + dt*128 + c]
    = a[dt*128 + p, nt*128 + c]."""
    out = np.empty((128, 12, 4, 128), a.dtype)
    for nt in range(12):
        for dt in range(4):
            out[:, nt, dt, :] = a[dt * 128:(dt + 1) * 128,
                                  nt * 128:(nt + 1) * 128]
    return np.ascontiguousarray(out.reshape(128, 6144))


def _coalesce(a):
    """(128*ND, W) -> (128, ND*W): [p, dt*W + c] = a[dt*128 + p, c]."""
    n, w = a.shape
    nd = n // 128
    return np.ascontiguousarray(
        a.reshape(nd, 128, w).transpose(1, 0, 2).reshape(128, nd * w)
    )


def _host_constants():
    freqs = np.exp(
        -np.log(10000.0) * np.arange(HALF, dtype=np.float32) / HALF
    )
    idx = np.arange(T, dtype=np.float32)
    ang = np.outer(freqs, idx)  # (HALF, T)
    sing = np.sin(ang).astype(np.float32)
    cosg = np.cos(ang).astype(np.float32)
    gt = _coalesce(np.concatenate([cosg, sing], axis=0)).astype(BF16)
    # fp8 DoubleRow pairing: gt8[p, a*1024 + t*512 + j] = trig_a[t*128+p, j]
    gt8 = np.empty((128, 2, 2, T), np.float32)
    for t in range(2):
        gt8[:, 0, t, :] = cosg[t * 128:(t + 1) * 128]
        gt8[:, 1, t, :] = sing[t * 128:(t + 1) * 128]
    gt8 = np.ascontiguousarray(gt8.reshape(128, 2048)).astype(F8)
    eye = np.eye(128, dtype=np.float32)
    msk = np.concatenate([NEG_BIG * eye, eye, eye], axis=1).astype(BF16)
    return gt, gt8, msk


def kernel(x, W_qkv, W_pos, pos_u, pos_v, W_out, b_out, log_one_div_by_tau):
    from concourse import bass_utils

    if "nc" not in _CACHE:
        _CACHE["nc"] = _build()
        _CACHE["consts"] = _host_constants()
    nc = _CACHE["nc"]
    gt, gt8, msk = _CACHE["consts"]

    x = np.asarray(x, np.float32)
    sml = np.zeros((128, 13), np.float32)
    for i in range(ND):
        sml[:, i] = np.asarray(pos_u, np.float32).reshape(D)[
            i * 128:(i + 1) * 128
        ]
        sml[:, 4 + i] = np.asarray(pos_v, np.float32).reshape(D)[
            i * 128:(i + 1) * 128
        ]
        sml[:, 8 + i] = np.asarray(b_out, np.float32).reshape(D)[
            i * 128:(i + 1) * 128
        ]
    sml[:, 12] = np.float32(np.asarray(log_one_div_by_tau).reshape(-1)[0])

    wqkv = _nmajor(np.asarray(W_qkv, np.float32)).astype(BF16)
    wpt = _coalesce(
        np.ascontiguousarray(np.asarray(W_pos, np.float32).T)
    ).astype(BF16)
    woutc = _coalesce(np.asarray(W_out, np.float32)).astype(BF16)

    def wqnb(nt):
        return wqkv[:, nt * 512:(nt + 1) * 512]

    par = np.concatenate(
        [
            wqnb(0), wqnb(4),                                  # A0
            np.ascontiguousarray(sml).view(BF16),
            wqnb(1), wqnb(5), wqnb(2), wqnb(6),                # A1
            wqnb(3), wqnb(7),                                  # A1b
            wpt,                                               # A2
            gt,                                                # A3
            np.ascontiguousarray(gt8).view(BF16),              # B
            msk,
            wqkv[:, 4096:6144],                                # C
            woutc,                                             # D
        ],
        axis=1,
    )
    assert par.shape == (128, 13722), par.shape

    in_maps = []
    for b in range(B):
        in_maps.append({
            "par": par,
            "xT": _coalesce(
                np.ascontiguousarray(x[:, b, :].T)
            ).astype(BF16),
        })

    _CACHE["last_in_maps"] = in_maps
    res = bass_utils.run_bass_kernel_spmd(nc, in_maps, core_ids=list(range(B)))
    out = np.empty((T, B, D), np.float32)
    for b in range(B):
        o = np.asarray(res.results[b]["out"], np.float32)  # (128, ND*T)
        out[:, b, :] = (
            o.reshape(128, ND, T).transpose(1, 0, 2).reshape(D, T).T
        )
    return out


# revision 55
# speedup vs baseline: 1.2050x; 1.0198x over previous
"""Trainium2 Bass kernel for Transformer-XL-style relative-position attention.

Problem (per reference):
  T=512 tokens, B=8 batch, D=512 model dim, H=8 heads, DH=64.
  energy = (q+u)@k^T + (q+v)@rpe^T(rel) ; rpe = sinusoidal(i-j) @ W_pos
  softmax over j (diag masked), out = (attn@v) @ W_out + b_out.

Strategy:
  - Data parallel over batch: core b computes batch element b end-to-end.
    No collectives needed.
  - The (t,t,d) rpe tensor is never materialized. Using
    sin((i-j)f) = sin(if)cos(jf) - cos(if)sin(jf) (and the cos analog),
    the BD term factorizes exactly into plain matmuls:
      P^T   = W_pos_h^T @ (q+v)^T            (per head, contraction 64)
      C1    = sin(if).P_sin + cos(if).P_cos  (elementwise, DVE)
      C2    = sin(if).P_cos - cos(if).P_sin
      BD^T  = G^T.T @ [C1;C2]  where G = [cos(jf) | sin(jf)] is constant.
  - The BD matmuls run in fp8e4 DoubleRow mode (2 k-tiles per
    instruction): half the PE column time of the bf16 version. C tiles
    are written fp8 directly by the DVE/GpSimd modulation ops.
  - Everything runs in feature-major ("transposed") layout (j on
    partitions): energies accumulate in PSUM together with the (q+u)k
    term and a single merged -BIG diagonal-mask matmul (strided 3-D
    output AP covers both j-tiles); one wide exp pass per j-pair
    produces attn^T.
  - V carries 64 ones-columns per head so the attn@v matmul emits the
    softmax denominator replicated on partitions 64:128; 1/den on DVE
    (reciprocal); one tensor_tensor divide.
  - Warm-up matmuls on a memset scratch tile run while the input DMAs
    land, ramping the PE DVFS clock before real work arrives. Inputs
    stream in dependency order in per-tile chunks.
  - bf16 matmul inputs (fp8 for BD), fp32 PSUM accumulation, bf16 out.
"""

import sys

sys.path.insert(0, "/opt/trn_rl_repo")

import numpy as np
import ml_dtypes

T, B, D, H = 512, 8, 512, 8
DH = D // H
HALF = D // 2
NT = T // 128          # 4 token tiles
ND = D // 128          # 4 feature tiles
NEG_BIG = -30000.0

BF16 = ml_dtypes.bfloat16
F8 = ml_dtypes.float8_e4m3

_CACHE = {}


def _patch_tile_drain():
    """walrus in this image rejects >1 sync-waits on one TPB_CTRL drain;
    split the TileContext tail-drain waits across several drains."""
    import concourse.tile as tile
    import concourse.mybir as mybir

    if getattr(tile.TileContext, "_drain_patched", False):
        return

    def _drain_and_barrier(self, tick_clock, wait_clock):
        from concourse.vector_clock import ScopedClock

        nc = self.nc
        drain_inst = nc.sync.drain()
        wait_clock.add_sem_waits(
            drain_inst.ins, ScopedClock({None: tick_clock.global_clock})
        )
        si = drain_inst.ins.sync_info
        waits = list(si.on_wait or [])
        if len(waits) > 1:
            si.on_wait[:] = waits[:1]
            for w in waits[1:]:
                extra = nc.sync.drain()
                extra.ins.sync_info = mybir.SyncInfo(on_wait=[w], on_update=[])

        nc.all_engine_barrier()
        assert self.sems is not None
        popped = nc._tile_sem_poison_stack.pop()
        assert popped is self._sem_poison
        nc.clear_and_free_semaphores(list(self.sems.allocated().values()))
        nc.all_engine_barrier()

    tile.TileContext._drain_and_barrier = _drain_and_barrier
    tile.TileContext._drain_patched = True


def _split_multi_waits(nc, limit=1):
    """This walrus build rejects >limit sync-waits on one instruction;
    hoist extra waits onto same-engine NoOp carriers placed just before."""
    import concourse.mybir as mybir

    ctr = [0]
    for f in nc.m.functions:
        for blk in f.blocks:
            new_insts = []
            for inst in blk.instructions:
                si = inst.sync_info
                waits = list(si.on_wait) if si and si.on_wait else []
                if len(waits) > limit:
                    for i in range(limit, len(waits), limit):
                        ctr[0] += 1
                        nop = mybir.InstNoOp(
                            name=f"waitnop{ctr[0]}", ins=[], outs=[]
                        )
                        nop.engine = inst.engine
                        nop.sync_info = mybir.SyncInfo(
                            on_wait=waits[i : i + limit], on_update=[]
                        )
                        new_insts.append(nop)
                    si.on_wait[:] = waits[:limit]
                new_insts.append(inst)
            blk.instructions[:] = new_insts


def _build():
    import concourse.bass as bass
    import concourse.mybir as mybir
    import concourse.tile as tile

    _patch_tile_drain()

    f32 = mybir.dt.float32
    bf16 = mybir.dt.bfloat16
    fp8 = mybir.dt.float8e4
    AF = mybir.ActivationFunctionType
    DR = mybir.MatmulPerfMode.DoubleRow

    nc = bass.Bass("TRN2", target_bir_lowering=True, debug=False, num_devices=B)

    with tile.TileContext(nc) as tc:
        # ---- DRAM parameters, packed into one tensor loaded as a few
        # chunked DMAs (each trigger costs ~650ns serialized on its
        # queue; triggers are split across the sync and scalar HWDGE
        # queues). Column layout (bf16 columns):
        #   A0 [    0: 1050): wqn0 | wqn4 | sml-as-bf16(26)
        #   A1 [ 1050: 3098): wqn1 | wqn5 | wqn2 | wqn6
        #   A1b[ 3098: 4122): wqn3 | wqn7
        #   A2 [ 4122: 6170): wpT(2048)
        #   A3 [ 6170: 8218): gt(2048)
        #   B  [ 8218: 9626): gt8-as-bf16(1024) | msk(384)
        #   C  [ 9626:11674): wqv(2048)
        #   D  [11674:13722): wout(2048)
        PCOLS = 13722
        xT_d = nc.dram_tensor("xT", [128, ND * T], bf16, kind="ExternalInput")
        par_d = nc.dram_tensor("par", [128, PCOLS], bf16, kind="ExternalInput")
        out_d = nc.dram_tensor("out", [128, ND * T], bf16, kind="ExternalOutput")

        # ---- static SBUF tiles -----------------------------------------
        with tc.tile_pool(name="static", bufs=1) as sp:
            warm = sp.tile([128, 640], bf16, name="warm")
            xtw = sp.tile([128, 4 * T], bf16, name="xtw")
            pA0 = sp.tile([128, 1050], bf16, name="pA0")
            pA1 = sp.tile([128, 3072], bf16, name="pA1")
            pA2 = sp.tile([128, 2048], bf16, name="pA2")
            pA3 = sp.tile([128, 2048], bf16, name="pA3")
            pB = sp.tile([128, 1408], bf16, name="pB")
            pC = sp.tile([128, 2048], bf16, name="pC")
            pD = sp.tile([128, 2048], bf16, name="pD")
            mb = sp.tile([128, 1], f32, name="mb")

            quT = [sp.tile([128, T], bf16, name=f"quT{i}") for i in range(ND)]
            qvT = [sp.tile([128, T], bf16, name=f"qvT{i}") for i in range(ND)]
            kT = [sp.tile([128, T], bf16, name=f"kT{i}") for i in range(ND)]
            vsb = [sp.tile([128, 8 * 128], bf16, name=f"v{i}") for i in range(NT)]
            avn = [sp.tile([128, T], bf16, name=f"avn{i}") for i in range(ND)]
            outw = sp.tile([128, ND * T], bf16, name="outw")

            # input DMAs in dependency order; fine-grained chunks so the
            # pipeline starts as soon as the first pieces land
            nc.sync.dma_start(xtw[:, 0:T], xT_d[:, 0:T])
            nc.sync.dma_start(pA0[:], par_d[:, 0:1050])
            nc.sync.dma_start(xtw[:, T:2 * T], xT_d[:, T:2 * T])
            nc.sync.dma_start(xtw[:, 2 * T:4 * T], xT_d[:, 2 * T:4 * T])
            nc.sync.dma_start(pA1[:, 0:1024], par_d[:, 1050:2074])
            nc.sync.dma_start(pA1[:, 1024:2048], par_d[:, 2074:3098])
            nc.sync.dma_start(pA1[:, 2048:3072], par_d[:, 3098:4122])
            nc.sync.dma_start(pA2[:, 0:1024], par_d[:, 4122:5146])
            nc.sync.dma_start(pA2[:, 1024:2048], par_d[:, 5146:6170])
            nc.sync.dma_start(pA3[:, 0:1024], par_d[:, 6170:7194])
            nc.sync.dma_start(pA3[:, 1024:2048], par_d[:, 7194:8218])
            nc.sync.dma_start(pB[:], par_d[:, 8218:9626])
            nc.sync.dma_start(pC[:, 0:1024], par_d[:, 9626:10650])
            nc.sync.dma_start(pC[:, 1024:2048], par_d[:, 10650:11674])
            nc.sync.dma_start(pD[:, 0:1024], par_d[:, 11674:12698])
            nc.sync.dma_start(pD[:, 1024:2048], par_d[:, 12698:13722])

            def xt(dt):
                return xtw[:, dt * T:(dt + 1) * T]

            wqn = {}
            for i, nt_ in enumerate((0, 4)):
                wqn[nt_] = pA0[:, i * 512:(i + 1) * 512]
            for i, nt_ in enumerate((1, 5, 2, 6, 3, 7)):
                wqn[nt_] = pA1[:, i * 512:(i + 1) * 512]
            wqv = pC[:, 0:2048]
            wptw = pA2[:, 0:2048]
            gtw = pA3[:, 0:2048]
            gt8w = pB[:, 0:1024].bitcast(fp8)
            msk = pB[:, 1024:1408]
            sml = pA0[:, 1024:1050].bitcast(f32)
            woutw = pD[:, 0:2048]

            sinw = gtw[:, 2 * T:4 * T]
            cosw = gtw[:, 0:2 * T]

            def wpT(dt, c0, c1):
                return wptw[:, dt * D + c0:dt * D + c1]

            def wout(dt, c0, c1):
                return woutw[:, dt * D + c0:dt * D + c1]

            def gt8(a, jt):
                # [128, 2, 128] DoubleRow stationary for trig bank a, j-tile jt
                return gt8w.rearrange(
                    "p (a t j) -> p a t j", a=2, t=2
                )[:, a, :, jt * 128:(jt + 1) * 128]

            dneg = msk[:, 0:128]
            eye2 = msk[:, 128:384]
            posu = [sml[:, i:i + 1] for i in range(ND)]
            posv = [sml[:, 4 + i:5 + i] for i in range(ND)]
            bout = [sml[:, 8 + i:9 + i] for i in range(ND)]
            ltau = sml[:, 12:13]

            # m = exp(ltau) * DH^-0.5  (log-space fold of the 1/8 scale)
            lnm = sp.tile([128, 1], f32, name="lnm")
            nc.vector.memset(lnm[:], float(np.log(DH ** -0.5)))
            nc.scalar.activation(mb[:], ltau, AF.Exp, bias=lnm[:], scale=1.0)

            with (
                tc.tile_pool(name="work", bufs=2) as wk,
                tc.tile_pool(name="ps", bufs=2, space="PSUM") as ps,
            ):
                # shared PSUM tags: acc (2x1 bank), p (1x2), e (2x2) = 8
                ps_p = ps_e = ps_av = ps

                # ---- PE warm-up: ramp the DVFS clock while inputs land.
                # warm is memset (no DMA dep) so these fire right after the
                # preamble barriers and keep the PE busy through the input
                # DMA window; outputs go to an e-tag PSUM tile nothing reads.
                nc.gpsimd.memset(warm[:], 0.0)
                wacc = ps.tile([128, T], f32, name="warm_ps", tag="e")
                for _ in range(9):
                    nc.tensor.matmul(
                        wacc[:],
                        warm[:, 0:128],
                        warm[:, 128:640],
                        start=True,
                        stop=True,
                        skip_group_check=True,
                    )

                def emit_qkv(ntile):
                    acc = ps.tile([128, T], f32, name="qkv_ps", tag="acc")
                    for dt in range(ND):
                        nc.tensor.matmul(
                            acc[:],
                            wqn[ntile][:, dt * 128:(dt + 1) * 128],
                            xt(dt),
                            start=(dt == 0),
                            stop=(dt == ND - 1),
                        )
                    if ntile < 4:
                        nc.scalar.activation(
                            quT[ntile][:], acc[:], AF.Identity,
                            bias=posu[ntile], scale=1.0,
                        )
                        nc.vector.tensor_scalar_add(
                            qvT[ntile][:], acc[:], posv[ntile]
                        )
                    else:
                        nc.scalar.copy(kT[ntile - 4][:], acc[:])

                def emit_v(it):
                    acc = ps.tile([128, D], f32, name="v_ps", tag="acc")
                    for dt in range(ND):
                        vrhs = wqv.rearrange(
                            "p (nt dt c) -> p nt dt c", nt=4, dt=4
                        )[:, :, dt, :]
                        nc.tensor.matmul(
                            acc[:],
                            xt(dt)[:, it * 128:(it + 1) * 128],
                            vrhs,
                            start=(dt == 0),
                            stop=(dt == ND - 1),
                        )
                    vview = vsb[it][:].rearrange("p (h c) -> p h c", c=128)
                    nc.scalar.copy(
                        vview[:, :, 0:64],
                        acc[:].rearrange("p (h c) -> p h c", c=64),
                    )
                    nc.gpsimd.memset(vview[:, :, 64:128], 1.0)

                # ---- per-head pipeline, software-pipelined ------------
                # P/modulation for head h+3 are interleaved into head h's
                # energy/attention matmuls so the PE never waits on the
                # DVE/GpSimd modulation chain.
                st = {}

                def emit_p_pair(h, gp):
                    hd_tile = h // 2
                    hd_off = (h % 2) * 64
                    qv_h = qvT[hd_tile][hd_off:hd_off + 64, :]
                    d = st.setdefault(h, {})
                    pacc = ps_p.tile([128, 2 * T], f32, name=f"p_ps{gp}_{h}",
                                     tag="p", bufs=1)
                    for g2 in range(2):
                        g = gp * 2 + g2
                        nc.tensor.matmul(
                            pacc[:, g2 * T:(g2 + 1) * T],
                            wpT(hd_tile, g * 128, (g + 1) * 128)[
                                hd_off:hd_off + 64, :
                            ],
                            qv_h,
                            start=True,
                            stop=True,
                            skip_group_check=True,
                        )
                    ptw = wk.tile([128, 2 * T], bf16, name=f"ptw{gp}_{h}",
                                  tag=f"ptw{gp}", bufs=4)
                    d[f"ptw{gp}"] = ptw
                    if gp == 0:
                        nc.scalar.copy(ptw[:], pacc[:])
                    else:
                        nc.vector.tensor_copy(ptw[:], pacc[:])

                def emit_mods(h):
                    # C1 = sin.Psin + cos.Pcos ; C2 = sin.Pcos - cos.Psin
                    # (fp8 outputs, DoubleRow rhs layout [p, f-half, i])
                    d = st[h]
                    psin, pcos = d["ptw0"], d["ptw1"]
                    ctw = [
                        wk.tile([128, 2 * T], fp8, name=f"ctw{g}_{h}",
                                tag=f"ctw{g}", bufs=4)
                        for g in range(2)
                    ]
                    d["ctw"] = ctw
                    ta = wk.tile([128, 2 * T], bf16, name=f"ta{h}", tag="ta",
                                 bufs=4)
                    tb = wk.tile([128, 2 * T], bf16, name=f"tb{h}", tag="tb",
                                 bufs=4)
                    nc.vector.tensor_mul(ta[:], sinw, psin[:])
                    nc.vector.tensor_mul(tb[:], cosw, pcos[:])
                    nc.vector.tensor_add(ctw[0][:], ta[:], tb[:])
                    ta2 = wk.tile([128, 2 * T], bf16, name=f"ta2{h}", tag="ta2",
                                  bufs=4)
                    tb2 = wk.tile([128, 2 * T], bf16, name=f"tb2{h}", tag="tb2",
                                  bufs=4)
                    nc.vector.tensor_mul(ta2[:], sinw, pcos[:])
                    nc.vector.tensor_mul(tb2[:], cosw, psin[:])
                    nc.gpsimd.tensor_sub(ctw[1][:], ta2[:], tb2[:])

                emit_qkv(0)
                emit_qkv(4)
                emit_p_pair(0, 0)
                emit_p_pair(0, 1)
                emit_qkv(1)
                emit_qkv(5)
                emit_mods(0)
                emit_p_pair(1, 0)
                emit_p_pair(1, 1)
                emit_qkv(2)
                emit_qkv(6)
                emit_mods(1)
                emit_p_pair(2, 0)
                emit_p_pair(2, 1)
                emit_qkv(3)
                emit_qkv(7)
                emit_mods(2)

                def emit_av_pair(h, attnTw, avacc, jts):
                    for jt in jts:
                        nc.tensor.matmul(
                            avacc[:],
                            vsb[jt][:, h * 128:(h + 1) * 128],
                            attnTw[jt // 2][:, (jt % 2) * T:(jt % 2 + 1) * T],
                            start=(jt == 0),
                            stop=(jt == NT - 1),
                            skip_group_check=True,
                        )

                for h in range(H):
                    hd_tile = h // 2
                    hd_off = (h % 2) * 64
                    qu_h = quT[hd_tile][hd_off:hd_off + 64, :]
                    ctw = st[h]["ctw"]

                    # energy^T per wide j-pair; one wide exp per pair
                    attnTw = [
                        wk.tile([128, 2 * T], bf16, name=f"atw{j}_{h}",
                                tag=f"atw{j}")
                        for j in range(2)
                    ]
                    for jp in range(2):
                        eacc = ps_e.tile([128, 2 * T], f32, name="e_ps", tag="e",
                                          bufs=2)
                        # K parts first (only need quT), then the fp8
                        # DoubleRow BD accumulations, merged mask last
                        for j2 in range(2):
                            jt = jp * 2 + j2
                            nc.tensor.matmul(
                                eacc[:, j2 * T:(j2 + 1) * T],
                                kT[hd_tile][hd_off:hd_off + 64,
                                            jt * 128:(jt + 1) * 128],
                                qu_h,
                                start=True,
                                stop=False,
                                skip_group_check=True,
                            )
                        for a in range(2):
                            crhs = ctw[a][:].rearrange(
                                "p (t n) -> p t n", t=2
                            )
                            for j2 in range(2):
                                jt = jp * 2 + j2
                                nc.tensor.matmul(
                                    eacc[:, j2 * T:(j2 + 1) * T],
                                    gt8(a, jt),
                                    crhs,
                                    start=False,
                                    stop=False,
                                    perf_mode=DR,
                                    skip_group_check=True,
                                )
                        # merged diag mask: one matmul, strided 3-D out
                        # covering cols {jp*256.. , 640+jp*256..}
                        from concourse.ap import AP

                        eap = eacc[:]
                        mout = AP(
                            eap.tensor,
                            eap.offset + jp * 256,
                            [[2 * T, 128], [640, 2], [1, 128]],
                        )
                        nc.tensor.matmul(
                            mout,
                            dneg,
                            eye2[:].rearrange("p (t c) -> p t c", t=2),
                            start=False,
                            stop=True,
                            skip_group_check=True,
                        )
                        nc.scalar.activation(
                            attnTw[jp][:], eacc[:], AF.Exp, bias=0.0, scale=mb[:],
                        )

                    # attn@v feature-major; ones rows give the denominator
                    # replicated on partitions 64:128. P matmuls for head
                    # h+3 are interleaved to cover the exp latency.
                    if h == 0:
                        for it_ in range(NT):
                            emit_v(it_)
                    avacc = ps_av.tile([128, T], f32, name="av_ps", tag="acc")
                    if h + 3 < H:
                        emit_p_pair(h + 3, 0)
                    emit_av_pair(h, attnTw, avacc, [0, 1])
                    if h + 3 < H:
                        emit_p_pair(h + 3, 1)
                    emit_av_pair(h, attnTw, avacc, [2, 3])
                    if h + 3 < H:
                        emit_mods(h + 3)
                    st.pop(h - 1, None)

                    lnden = wk.tile([64, T], f32, name="lnden", tag="lnden")
                    rdb = wk.tile([64, T], f32, name="rdb", tag="rdb")
                    nc.scalar.activation(
                        lnden[:], avacc[64:128, :], AF.Ln, bias=0.0, scale=1.0
                    )
                    nc.scalar.activation(
                        rdb[:], lnden[:], AF.Exp, bias=0.0, scale=-1.0
                    )
                    nc.vector.tensor_mul(
                        avn[hd_tile][hd_off:hd_off + 64, :],
                        avacc[0:64, :],
                        rdb[:],
                    )

                    if h == 5:
                        # W_out partials for ot 0,1 over d-tiles 0,1
                        # (avn[0..2] are complete after this head; dt=2
                        # is deferred to h==6 to fill that head's gap)
                        st["oacc01"] = ps.tile(
                            [128, 2 * T], f32, name="oacc01", tag="p", bufs=1
                        )
                        for dt in range(2):
                            for o2 in range(2):
                                nc.tensor.matmul(
                                    st["oacc01"][:, o2 * T:(o2 + 1) * T],
                                    wout(dt, o2 * 128, (o2 + 1) * 128),
                                    avn[dt][:],
                                    start=(dt == 0),
                                    stop=False,
                                    skip_group_check=True,
                                )
                    if h == 6:
                        for o2 in range(2):
                            nc.tensor.matmul(
                                st["oacc01"][:, o2 * T:(o2 + 1) * T],
                                wout(2, o2 * 128, (o2 + 1) * 128),
                                avn[2][:],
                                start=False,
                                stop=False,
                                skip_group_check=True,
                            )
                    if h == 7:
                        st["oacc23"] = ps.tile(
                            [128, 2 * T], f32, name="oacc23", tag="e", bufs=2
                        )
                        for dt in range(3):
                            for o2 in range(2):
                                nc.tensor.matmul(
                                    st["oacc23"][:, o2 * T:(o2 + 1) * T],
                                    wout(dt, (2 + o2) * 128, (3 + o2) * 128),
                                    avn[dt][:],
                                    start=(dt == 0),
                                    stop=False,
                                    skip_group_check=True,
                                )
                # zero-contribution matmuls (warm is memset 0) keep the PE
                # clock at max p-state through the last head's den chain
                for _ in range(4):
                    nc.tensor.matmul(
                        st["oacc23"][:, 0:T],
                        warm[:, 0:128],
                        warm[:, 128:640],
                        start=False,
                        stop=False,
                        skip_group_check=True,
                    )

                # ---- output projection: finish dt=3 and write out -----
                for pair, tname in ((0, "oacc01"), (1, "oacc23")):
                    oacc = st[tname]
                    for o2 in range(2):
                        ot = pair * 2 + o2
                        nc.tensor.matmul(
                            oacc[:, o2 * T:(o2 + 1) * T],
                            wout(3, ot * 128, (ot + 1) * 128),
                            avn[3][:],
                            start=False,
                            stop=True,
                            skip_group_check=True,
                        )
                        nc.vector.tensor_scalar_add(
                            outw[:, ot * T:(ot + 1) * T],
                            oacc[:, o2 * T:(o2 + 1) * T],
                            bout[ot],
                        )
                        nc.sync.dma_start(
                            out_d[:, ot * T:(ot + 1) * T],
                            outw[:, ot * T:(ot + 1) * T],
                        )

    _split_multi_waits(nc)
    return nc


def _nmajor(a):
    """(512, 1536) -> (128, 12*4*128): [p, nt*512 + dt*128 + c]
    = a[dt*128 + p, nt*128 + c]."""
    out = np.empty((128, 12, 4, 128), a.dtype)
    for nt in range(12):
        for dt in range(4):
            out[:, nt, dt, :] = a[dt * 128:(dt + 1) * 128,
                                  nt * 128:(nt + 1) * 128]
    return np.ascontiguousarray(out.reshape(128, 6144))


def _coalesce(a):
    """(128*ND, W) -> (128, ND*W): [p, dt*W + c] = a[dt*128 + p, c]."""
    n, w = a.shape
    nd = n // 128
    return np.ascontiguousarray(
        a.reshape(nd, 128, w).transpose(1, 0, 2).reshape(128, nd * w)
    )


def _host_constants():
    freqs = np.exp(
        -np.log(10000.0) * np.arange(HALF, dtype=np.float32) / HALF
    )
    idx = np.arange(T, dtype=np.float32)
    ang = np.outer(freqs, idx)  # (HALF, T)
    sing = np.sin(ang).astype(np.float32)
    cosg = np.cos(ang).astype(np.float32)
    gt = _coalesce(np.concatenate([cosg, sing], axis=0)).astype(BF16)
    # fp8 DoubleRow pairing: gt8[p, a*1024 + t*512 + j] = trig_a[t*128+p, j]
    gt8 = np.empty((128, 2, 2, T), np.float32)
    for t in range(2):
        gt8[:, 0, t, :] = cosg[t * 128:(t + 1) * 128]
        gt8[:, 1, t, :] = sing[t * 128:(t + 1) * 128]
    gt8 = np.ascontiguousarray(gt8.reshape(128, 2048)).astype(F8)
    eye = np.eye(128, dtype=np.float32)
    msk = np.concatenate([NEG_BIG * eye, eye, eye], axis=1).astype(BF16)
    return gt, gt8, msk


def kernel(x, W_qkv, W_pos, pos_u, pos_v, W_out, b_out, log_one_div_by_tau):
    from concourse import bass_utils

    if "nc" not in _CACHE:
        _CACHE["nc"] = _build()
        _CACHE["consts"] = _host_constants()
    nc = _CACHE["nc"]
    gt, gt8, msk = _CACHE["consts"]

    x = np.asarray(x, np.float32)
    sml = np.zeros((128, 13), np.float32)
    for i in range(ND):
        sml[:, i] = np.asarray(pos_u, np.float32).reshape(D)[
            i * 128:(i + 1) * 128
        ]
        sml[:, 4 + i] = np.asarray(pos_v, np.float32).reshape(D)[
            i * 128:(i + 1) * 128
        ]
        sml[:, 8 + i] = np.asarray(b_out, np.float32).reshape(D)[
            i * 128:(i + 1) * 128
        ]
    sml[:, 12] = np.float32(np.asarray(log_one_div_by_tau).reshape(-1)[0])

    wqkv = _nmajor(np.asarray(W_qkv, np.float32)).astype(BF16)
    wpt = _coalesce(
        np.ascontiguousarray(np.asarray(W_pos, np.float32).T)
    ).astype(BF16)
    woutc = _coalesce(np.asarray(W_out, np.float32)).astype(BF16)

    def wqnb(nt):
        return wqkv[:, nt * 512:(nt + 1) * 512]

    par = np.concatenate(
        [
            wqnb(0), wqnb(4),                                  # A0
            np.ascontiguousarray(sml).view(BF16),
            wqnb(1), wqnb(5), wqnb(2), wqnb(6),                # A1
            wqnb(3), wqnb(7),                                  # A1b
            wpt,                                               # A2
            gt,                                                # A3
            np.ascontiguousarray(gt8).view(BF16),              # B
            msk,
            wqkv[:, 4096:6144],                                # C
            woutc,                                             # D
        ],
        axis=1,
    )
    assert par.shape == (128, 13722), par.shape

    in_maps = []
    for b in range(B):
        in_maps.append({
            "par": par,
            "xT": _coalesce(
                np.ascontiguousarray(x[:, b, :].T)
            ).astype(BF16),
        })

    _CACHE["last_in_maps"] = in_maps
    res = bass_utils.run_bass_kernel_spmd(nc, in_maps, core_ids=list(range(B)))
    out = np.empty((T, B, D), np.float32)
    for b in range(B):
        o = np.asarray(res.results[b]["out"], np.float32)  # (128, ND*T)
        out[:, b, :] = (
            o.reshape(128, ND, T).transpose(1, 0, 2).reshape(D, T).T
        )
    return out
